# revision 13
# baseline (speedup 1.0000x reference)
"""MoE brute-force linear: o[t] = weight[gate[t]] @ inp[t].

Strategy: expert-parallel over 8 NeuronCores (2 experts/core), fp16,
token-moving matmuls.
  Host: stable-sort tokens by gate, pair the largest expert with the
  smallest on each core (slot 0 / slot 1), pad each slot's token block
  only to the max count over cores (SPMD uniformity) — no 128-group
  rounding. Pre-transpose activations and weights to fp16.
  Device: per (jt, kt) the stationary operand is a 128x128 weight tile
  W_e[kt, jt] and the moving operand is the token chunk (<=512 tokens,
  fp16 runs 1 cycle/row at any moving length), accumulating K=1024 over
  8 PSUM passes. Output is produced as y^T tiles [128 j, 8 jt, c tok]
  and untransposed on the host.
  DMA streams are decoupled: x loads on the SP HWDGE ring, weight loads
  on SWDGE (gpsimd), y stores on the ACT HWDGE ring.
"""

import numpy as np

BATCH = 8192
D = 1024
N_EXPERT = 16
N_CORES = 8
E_PER_CORE = N_EXPERT // N_CORES  # 2
KT = D // 128  # 8 contraction k-tiles
JT = D // 128  # 8 output j-tiles
CHUNK_MAX = 512  # PSUM bank limit (512 f32 per partition)

LAST_RESULT = None  # BassKernelResults of the most recent run


def _split_multiwait(nc):
    """Split every >1-sem-wait instruction into single-wait NoOps placed
    just before it on the same engine (this walrus rejects multi-wait
    CTRL instructions)."""
    import concourse.mybir as mybir

    for fn in nc.m.functions:
        for bb in fn.blocks:
            il = bb.instructions
            i = 0
            while i < len(il):
                ins = il[i]
                si = getattr(ins, "sync_info", None)
                if si is not None and len(si.on_wait) > 1:
                    waits = list(si.on_wait)
                    ins.sync_info = mybir.SyncInfo(
                        on_wait=[waits[-1]], on_update=list(si.on_update)
                    )
                    nops = [
                        mybir.InstNoOp(
                            name=f"{ins.name}-wsplit{k}",
                            engine=ins.engine,
                            sync_info=mybir.SyncInfo(on_wait=[w], on_update=[]),
                            bass_nofuse=True,
                        )
                        for k, w in enumerate(waits[:-1])
                    ]
                    il[i:i] = nops
                    i += len(nops)
                i += 1
    return nc


def _chunks_of(cap):
    """Split cap tokens into the fewest even chunks of at most CHUNK_MAX
    (fewer matmul instructions; even split keeps every chunk >=128 so
    LDWEIGHTS stays hidden under the moving stream)."""
    cap = max(int(cap), 1)
    n = -(-cap // CHUNK_MAX)
    base, rem = divmod(cap, n)
    return [base + 1] * rem + [base] * (n - rem)


def _plan(counts):
    """Assign experts to (core, slot): slot 0 takes the 8 largest experts,
    slot 1 the 8 smallest, pairing rank c with rank 15-c. Slot capacity =
    max count over cores (SPMD uniform); chunk lists split each capacity
    into <=512-token pieces."""
    rank = np.argsort(-counts, kind="stable")
    expert_of = [[int(rank[c]), int(rank[N_EXPERT - 1 - c])] for c in range(N_CORES)]
    caps, chunk_lists = [], []
    for i in range(E_PER_CORE):
        cap = max(max(int(counts[expert_of[c][i]]) for c in range(N_CORES)), 1)
        caps.append(cap)
        chunk_lists.append(_chunks_of(cap))
    return expert_of, caps, chunk_lists


def _build_program(plan, reps=1, loop_reps=None):
    """loop_reps: if set, wrap `reps` python-unrolled kernel bodies in a
    For_i hardware loop executing loop_reps iterations (timing only —
    total kernel executions = reps * loop_reps)."""
    expert_of, caps, chunk_lists = plan
    import contextlib
    import concourse.bass as bass
    import concourse.tile as tile
    import concourse.mybir as mybir

    f16 = mybir.dt.float16
    f32 = mybir.dt.float32

    nc = bass.Bass()
    # wT[i, kt, kp, j] = W_e[j, kt*128+kp]  (fp16)
    wT = nc.dram_tensor("wT", [E_PER_CORE, KT, 128, D], f16, kind="ExternalInput")
    xs, ys = [], []
    for i in range(E_PER_CORE):
        xs.append(
            [
                nc.dram_tensor(f"x{i}_{k}", [128, KT, c], f16, kind="ExternalInput")
                for k, c in enumerate(chunk_lists[i])
            ]
        )
        ys.append(
            [
                nc.dram_tensor(f"y{i}_{k}", [128, JT, c], f32, kind="ExternalOutput")
                for k, c in enumerate(chunk_lists[i])
            ]
        )

    with tile.TileContext(nc) as tc:
        with (
            tc.tile_pool(name="wpool", bufs=3 * KT) as wpool,
            tc.tile_pool(name="xpool", bufs=8) as xpool,
            tc.tile_pool(name="opool", bufs=6) as opool,
            tc.tile_pool(name="pspool", bufs=8, space="PSUM") as pspool,
            contextlib.nullcontext(),
        ):
            wt_pre = None
            if loop_reps:
                # SWDGE (gpsimd) DMA inside For_i breaks walrus codegen, and
                # in steady state weights are SBUF-resident anyway: load all
                # slots' weights once, before the loop.
                wt_pre = []
                for i in range(E_PER_CORE):
                    tiles = []
                    for kt in range(KT):
                        w_tile = wpool.tile([128, D], f16, tag="w")
                        nc.gpsimd.dma_start(out=w_tile[:], in_=wT[i, kt])
                        tiles.append(w_tile)
                    wt_pre.append(tiles)
            loop_cm = tc.For_i(0, loop_reps) if loop_reps else contextlib.nullcontext()
            with loop_cm:
              for rep in range(reps):
                for i in range(E_PER_CORE):
                    if wt_pre is not None:
                        wt = wt_pre[i]
                    else:
                        wt = []
                        for kt in range(KT):
                            w_tile = wpool.tile([128, D], f16, tag="w")
                            nc.gpsimd.dma_start(out=w_tile[:], in_=wT[i, kt])
                            wt.append(w_tile)
                    for k, c in enumerate(chunk_lists[i]):
                        first_chunk = rep == 0 and i == 0 and k == 0
                        xt = xpool.tile([128, KT, c], f16, tag="x")
                        if first_chunk:
                            # split the first chunk's x load per k-tile so the
                            # kt-major compute below starts after 1/8 of it
                            for kt in range(KT):
                                nc.sync.dma_start(
                                    out=xt[:, kt], in_=xs[i][k][:, kt]
                                )
                        else:
                            nc.sync.dma_start(out=xt[:], in_=xs[i][k][:])
                        ot = opool.tile([128, JT, c], f32, tag="o")
                        if first_chunk:
                            # cold start: kt-major so compute begins after
                            # the FIRST weight tile lands, not all eight
                            pss = [
                                pspool.tile(
                                    [128, CHUNK_MAX], f32, tag="ps",
                                    name=f"ps_cold{jt}",
                                )
                                for jt in range(JT)
                            ]
                            for kt in range(KT):
                                for jt in range(JT):
                                    nc.tensor.matmul(
                                        pss[jt][:, :c],
                                        lhsT=wt[kt][:, jt * 128 : (jt + 1) * 128],
                                        rhs=xt[:, kt],
                                        start=(kt == 0),
                                        stop=(kt == KT - 1),
                                    )
                            for jt in range(JT):
                                nc.vector.tensor_copy(ot[:, jt], pss[jt][:, :c])
                                yq = nc.scalar if (loop_reps or jt % 2 == 0) else nc.gpsimd
                                yq.dma_start(out=ys[i][k][:, jt], in_=ot[:, jt])
                        else:
                            for jt in range(JT):
                                ps = pspool.tile([128, CHUNK_MAX], f32, tag="ps")
                                for kt in range(KT):
                                    nc.tensor.matmul(
                                        ps[:, :c],
                                        lhsT=wt[kt][:, jt * 128 : (jt + 1) * 128],
                                        rhs=xt[:, kt],
                                        start=(kt == 0),
                                        stop=(kt == KT - 1),
                                    )
                                nc.vector.tensor_copy(ot[:, jt], ps[:, :c])
                                # store each j-tile as soon as its copy lands;
                                # split y across ACT/SWDGE rings (SWDGE DMA is
                                # illegal inside For_i, so loop mode uses ACT)
                                yq = nc.scalar if (loop_reps or jt % 2 == 0) else nc.gpsimd
                                yq.dma_start(out=ys[i][k][:, jt], in_=ot[:, jt])
    _split_multiwait(nc)
    return nc


def _prep_inputs(inp, gate, weight):
    inp = np.ascontiguousarray(np.asarray(inp), dtype=np.float32)
    gate = np.asarray(gate).astype(np.int64)
    weight = np.ascontiguousarray(np.asarray(weight), dtype=np.float32)

    order = np.argsort(gate, kind="stable")
    counts = np.bincount(gate[order], minlength=N_EXPERT)
    starts = np.zeros(N_EXPERT + 1, dtype=np.int64)
    np.cumsum(counts, out=starts[1:])
    plan = _plan(counts)
    expert_of, caps, chunk_lists = plan

    x_sorted = inp[order].astype(np.float16)  # [B, D]
    w16 = weight.astype(np.float16)

    in_maps = []
    for c in range(N_CORES):
        m = {}
        wT = np.empty((E_PER_CORE, KT, 128, D), dtype=np.float16)
        for i in range(E_PER_CORE):
            e = expert_of[c][i]
            n_e = int(counts[e])
            xe = np.zeros((caps[i], D), dtype=np.float16)
            xe[:n_e] = x_sorted[starts[e] : starts[e] + n_e]
            off = 0
            for k, ck in enumerate(chunk_lists[i]):
                # [ck, D] -> [ck(t), KT, 128(kp)] -> [kp, kt, t]
                m[f"x{i}_{k}"] = np.ascontiguousarray(
                    xe[off : off + ck].reshape(ck, KT, 128).transpose(2, 1, 0)
                )
                off += ck
            # W_e [D_out, D_in] -> transpose -> [KT, 128(kp), D_out]
            wT[i] = w16[e].T.reshape(KT, 128, D)
        m["wT"] = wT
        in_maps.append(m)
    return in_maps, plan, order, counts, starts


def _gather_output(results, plan, order, counts, starts):
    expert_of, caps, chunk_lists = plan
    out = np.empty((BATCH, D), dtype=np.float32)
    for c in range(N_CORES):
        for i in range(E_PER_CORE):
            e = expert_of[c][i]
            n_e = int(counts[e])
            if not n_e:
                continue
            idx = order[starts[e] : starts[e] + n_e]
            off = 0
            for k, ck in enumerate(chunk_lists[i]):
                take = min(ck, n_e - off)
                if take <= 0:
                    break
                yc = results[c][f"y{i}_{k}"]  # [128(jp), JT, ck]
                rows = yc.transpose(2, 1, 0).reshape(ck, D)
                out[idx[off : off + take]] = rows[:take]
                off += ck
    return out


def kernel(inp, gate, weight):
    global LAST_RESULT
    from concourse.bass_utils import run_bass_kernel_spmd

    in_maps, plan, order, counts, starts = _prep_inputs(inp, gate, weight)
    nc = _build_program(plan)

    last_err = None
    for attempt in range(3):
        try:
            res = run_bass_kernel_spmd(nc, in_maps, core_ids=list(range(N_CORES)))
            break
        except Exception as exc:  # transient NRT device errors: retry
            last_err = exc
            import time

            time.sleep(2.0 * (attempt + 1))
    else:
        raise last_err
    LAST_RESULT = res

    return _gather_output(res.results, plan, order, counts, starts)
